# revision 1
# baseline (speedup 1.0000x reference)
"""Trainium2 Bass kernel for the multi-agent attention critic.

Strategy: data-parallel over the batch dim across 8 NeuronCores
(4096 samples/core). Inside each core everything is computed
feature-major ([feature, batch] tiles, batch on the free dim) in bf16
with fp32 PSUM accumulation:

  - one merged input DMA per tile: [82, 8, Bt] = 7 other-agent blocks
    plus the (padded) self-branch input in block 7.
  - self branch:  x1 = relu(W1.T xT + b1); x2 = relu(W2.T x1 + b2)
  - per agent a:  enc_a = relu(We_a.T inps_a + be_a)
                  keys_a = Wk.T enc_a (PSUM), vals_a = relu(Wv.T enc_a + bv)
                  prod_a = sel * keys_a (DVE); an indicator matmul per
                  agent accumulates prod's per-head sums into logits row
                  7h + a of a compact [28, Bt] PSUM tile (h-major).
  - softmax: ee = exp(L); Z = segsum via indicator matmul; ls = ln(Z);
    L -= bcast(ls) via a -1-indicator matmul (bf16); w = exp(L).
    exp/ln/relu/copy all live in one ACT table set.
  - ov: per agent, wbc_a = bcast(w rows) via indicator matmul (PSUM),
    pa_a = vals_a * wbc_a (DVE), and the agent-sum is folded into the
    w3_others matmul by accumulating into the x3 PSUM tile.
  - out = Wout.T relu(x3s + x3o) + bout

PSUM->SBUF evacuations are split between ACT and DVE by the EVAC knob
so both engines carry ~equal load (ACT ~620 ns/op, DVE ~660 ns/op at
Bt=512; DVE also owns the 14 tensor-tensor products). GPSIMD is left
idle on purpose: its tensor ops measure ~2.3x the simulator cost on
real silicon and it cannot read PSUM, which every remaining op touches.
DMA partition-broadcast was tried for the wbc step and measured ~35 us
per [4->128, 7*Bt] broadcast on hardware (all descriptors read from 4
source partitions -> source-port bound), so the broadcast stays on PE.

The tile loop is software-pipelined: tile t's softmax (b1) is emitted
inside tile t+1's stage A, and tile t's ov/head phase (b2) two tiles
later, so the long cross-engine chains hide behind two tiles of
independent matmul work. The 1/sqrt(d) attention scale is folded into
Wq on the host; all constants ship in two packed DMAs.
"""

import numpy as np
import ml_dtypes

B = 32768
NA = 8
A = NA - 1
OBS = 64
ACTD = 14
OTH_IN = 82
H_SELF = 64
H_OTH = 128
H2 = 64
HEADS = 4
AD = H_OTH // HEADS  # 32
NCORES = 8
BC = B // NCORES     # 4096 samples per core
BT = 512             # batch tile (free dim per matmul)
NT = BC // BT        # 8 tiles per core
X_IN = OBS + ACTD    # 78
A_SPLIT = 3          # agents emitted in stage-A front vs back

BF16 = ml_dtypes.bfloat16

_CACHE = {}

# ---- tuning knobs ----------------------------------------------------
# engine for each PSUM->SBUF evacuation: "act" or "dve"
EVAC = {
    "x1": "act", "x2": "act", "sel": "act",
    "enc": ["act", "act", "act", "act", "act", "act", "dve"],
    "vals": ["act", "act", "act", "act", "act", "dve", "dve"],
}
# engines for the 4 S-add tree ops (q1, q2, L1pair, L2): "pool" or "dve"
SADD = ["dve", "dve", "dve", "dve"]
# engines for the final sum join and the 1/Z scale: "pool" or "dve"
SMISC = {"s_sum": "dve", "sn": "dve"}
# engines for the 4 pa products (3 pairs + 1 single)
PA = ["dve", "dve", "dve", "dve"]


def _split_sync_waits(nc):
    """This walrus build rejects instructions carrying too many sem-wait
    conditions ("Too many sync wait commands"): 2 for compute instructions,
    1 for CTRL ops (Drain etc). Split extra waits onto preceding same-engine
    NOPs — engines execute their own stream in order, so a wait on an
    earlier NOP is equivalent."""
    import concourse.mybir as mybir

    n_added = 0
    for fn in nc.m.functions:
        for bb in fn.blocks:
            out = []
            for inst in bb.instructions:
                max_waits = 1
                si = inst.sync_info
                if si is not None and si.on_wait and len(si.on_wait) > max_waits:
                    waits = list(si.on_wait)
                    si.on_wait = waits[:max_waits]
                    rest = waits[max_waits:]
                    for k in range(0, len(rest), 1):
                        nop = mybir.InstNoOp(
                            name=f"{inst.name}-ws{k}", ins=[], outs=[],
                            bass_nofuse=True)
                        nop.engine = inst.engine
                        nop.sync_info = mybir.SyncInfo(
                            on_wait=[rest[k]], on_update=[])
                        out.append(nop)
                        n_added += 1
                out.append(inst)
            bb.instructions[:] = out
    return n_added


LROWS = 28   # logits PSUM rows: row of (h, a) = 7h + a (h-major)
AB = 7       # agent blocks in the broadcast free dim

# Packed-constant layouts: (name, rows, cols). Offsets 4-col aligned.
_CONSTS_BF16 = [
    ("w1", 78, 64), ("w2", 64, 64), ("w3s", 64, 64), ("wq", 64, 128),
    ("we", 82, 7 * 128), ("wk", 128, 128), ("wv", 128, 128),
    ("w3o", 128, 64), ("wout", 64, 1), ("sind", 128, 7 * LROWS),
    ("t4", LROWS, 4), ("negind", 4, LROWS), ("wl", LROWS, 7 * 128),
]
_CONSTS_F32 = [
    ("b1", 64, 1), ("b2", 64, 1), ("be", 128, 7), ("bv", 128, 1),
    ("bout", 1, 1),
]


def _pack_layout(spec):
    off, w = {}, 0
    for name, rows, cols in spec:
        off[name] = w
        w += (cols + 3) // 4 * 4
    return off, w


def _const_view(spec, off, name):
    for n, rows, cols in spec:
        if n == name:
            return rows, off[name], off[name] + cols
    raise KeyError(name)


def _indicator_constants():
    """Constant indicator matrices for the attention bookkeeping.
    Logits row of (h, a) = 8h + a (h-major so that the [32, BT] logits
    tile reshapes to the broadcast-ready [4, 8*BT] layout with one
    plain DMA). All seven segred matmuls write the same 32 rows,
    accumulating: agent a's lhsT puts head indicators in its own 4 rows
    and zeros elsewhere."""
    sind = np.zeros((H_OTH, A, LROWS), dtype=BF16)
    for hd in range(H_OTH):
        for a in range(A):
            sind[hd, a, A * (hd // AD) + a] = 1.0
    # t4[p, h]: segsum lhsT — sums ee rows of head h over agents.
    t4 = np.zeros((LROWS, HEADS), dtype=BF16)
    for a in range(A):
        for h in range(HEADS):
            t4[A * h + a, h] = 1.0
    # negind[h, p]: subtract-broadcast lhsT — L[p] -= ls[h(p)].
    negind = np.zeros((HEADS, LROWS), dtype=BF16)
    for a in range(A):
        for h in range(HEADS):
            negind[h, A * h + a] = -1.0
    # wl[k, a, p]: per-agent broadcast lhsT — wbc_a[p] = w[7*(p//32)+a].
    wl = np.zeros((LROWS, A, H_OTH), dtype=BF16)
    for p in range(H_OTH):
        for a in range(A):
            wl[A * (p // AD) + a, a, p] = 1.0
    return sind, t4, negind, wl


def _build_nc(reps=1):
    import concourse.bass as bass
    import concourse.mybir as mybir
    import concourse.tile as tile
    from contextlib import ExitStack

    dt = mybir.dt
    AF = mybir.ActivationFunctionType
    ALU = mybir.AluOpType

    nc = bass.Bass("TRN2", target_bir_lowering=False, debug=False)

    # ---- DRAM I/O ------------------------------------------------------
    # merged input: blocks 0..6 = other-agent inputs, block 7 rows
    # 0..77 = the self-branch input (padded to 82 rows)
    ot = nc.dram_tensor("ot", [OTH_IN, NA, BC], dt.bfloat16,
                        kind="ExternalInput")
    cb_off, cb_w = _pack_layout(_CONSTS_BF16)
    cf_off, cf_w = _pack_layout(_CONSTS_F32)
    cb = nc.dram_tensor("cb", [128, cb_w], dt.bfloat16, kind="ExternalInput")
    cf = nc.dram_tensor("cf", [128, cf_w], dt.float32, kind="ExternalInput")

    out_d = nc.dram_tensor("out", [1, BC], dt.float32, kind="ExternalOutput")

    with tile.TileContext(nc) as tc, ExitStack() as ctx:
        singles = ctx.enter_context(tc.tile_pool(name="singles", bufs=1))

        s_cb = singles.tile([128, cb_w], dt.bfloat16, name="s_cb")
        nc.sync.dma_start(out=s_cb, in_=cb.ap())
        s_cf = singles.tile([128, cf_w], dt.float32, name="s_cf")
        nc.sync.dma_start(out=s_cf, in_=cf.ap())

        def bslice(name, rows=None):
            r, c0, c1 = _const_view(_CONSTS_BF16, cb_off, name)
            return s_cb[: (rows or r), c0:c1]

        def fslice(name, rows=None):
            r, c0, c1 = _const_view(_CONSTS_F32, cf_off, name)
            return s_cf[: (rows or r), c0:c1]

        s_w1 = bslice("w1")
        s_w2 = bslice("w2")
        s_w3s = bslice("w3s")
        s_wq = bslice("wq")
        s_wk = bslice("wk")
        s_wv = bslice("wv")
        s_w3o = bslice("w3o")
        s_wout = bslice("wout")
        s_t4 = bslice("t4")
        s_b1 = fslice("b1")
        s_b2 = fslice("b2")
        s_be = fslice("be")
        s_bv = fslice("bv")
        s_bout = fslice("bout")
        s_negind = bslice("negind")
        _, we0, _ = _const_view(_CONSTS_BF16, cb_off, "we")
        _, si0, _ = _const_view(_CONSTS_BF16, cb_off, "sind")
        _, wl0, _ = _const_view(_CONSTS_BF16, cb_off, "wl")

        def s_wl(a):
            return s_cb[:LROWS, wl0 + a * H_OTH: wl0 + (a + 1) * H_OTH]

        def s_we(a):
            return s_cb[:OTH_IN, we0 + a * H_OTH: we0 + (a + 1) * H_OTH]

        def s_sind(a):
            return s_cb[:, si0 + a * LROWS: si0 + (a + 1) * LROWS]

        # SBUF working pools
        p_ot = ctx.enter_context(tc.tile_pool(name="p_ot", bufs=3))
        p_act = ctx.enter_context(tc.tile_pool(name="p_act", bufs=3))
        p_enc = ctx.enter_context(tc.tile_pool(name="p_enc", bufs=3))
        p_prod = ctx.enter_context(tc.tile_pool(name="p_prod", bufs=3))
        p_vals = ctx.enter_context(tc.tile_pool(name="p_vals", bufs=3))
        p_sm = ctx.enter_context(tc.tile_pool(name="p_sm", bufs=3))
        p_pa = ctx.enter_context(tc.tile_pool(name="p_pa", bufs=3))

        outs_all = singles.tile([1, BC], dt.float32, name="outs_all")

        # PSUM pools (8 banks total)
        ps1 = ctx.enter_context(tc.tile_pool(name="ps1", bufs=2, space="PSUM"))
        pskv = ctx.enter_context(tc.tile_pool(name="pskv", bufs=2, space="PSUM"))
        psl = ctx.enter_context(tc.tile_pool(name="psl", bufs=2, space="PSUM"))
        pswb = ctx.enter_context(tc.tile_pool(name="pswb", bufs=1, space="PSUM"))
        psx3 = ctx.enter_context(tc.tile_pool(name="psx3", bufs=1, space="PSUM"))

        NTOT = NT * reps

        def evac(engine, out, in_, bias=None, relu=False):
            """PSUM->SBUF evacuation on the chosen engine."""
            if engine == "act":
                if relu:
                    nc.scalar.activation(out, in_, AF.Relu, bias=bias)
                elif bias is not None:
                    nc.scalar.activation(out, in_, AF.Identity, bias=bias)
                else:
                    nc.scalar.activation(out, in_, AF.Copy)
            else:
                if relu:
                    nc.vector.tensor_scalar(
                        out=out, in0=in_, scalar1=bias, scalar2=0.0,
                        op0=ALU.add, op1=ALU.max)
                elif bias is not None:
                    nc.vector.tensor_scalar(
                        out=out, in0=in_, scalar1=bias, scalar2=None,
                        op0=ALU.add)
                else:
                    nc.vector.tensor_copy(out, in_)

        def stage_a(t):
            """Inputs, self branch, front agents."""
            b0 = (t % NT) * BT
            ots = p_ot.tile([OTH_IN, NA, BT], dt.bfloat16, tag="ots")
            nc.sync.dma_start(out=ots, in_=ot.ap()[:, :, b0:b0 + BT])
            xts = ots[:X_IN, A, :]

            x1p = ps1.tile([H_SELF, BT], dt.float32, tag="m", name="x1p")
            nc.tensor.matmul(x1p, s_w1, xts, start=True, stop=True)
            x1 = p_act.tile([H_SELF, BT], dt.bfloat16, tag="x1")
            evac(EVAC["x1"], x1, x1p, bias=s_b1, relu=True)

            x2p = ps1.tile([H_SELF, BT], dt.float32, tag="m", name="x2p")
            nc.tensor.matmul(x2p, s_w2, x1, start=True, stop=True)
            x2 = p_act.tile([H_SELF, BT], dt.bfloat16, tag="x2")
            evac(EVAC["x2"], x2, x2p, bias=s_b2, relu=True)

            selp = ps1.tile([H_OTH, BT], dt.float32, tag="m", name="selp")
            nc.tensor.matmul(selp, s_wq, x1, start=True, stop=True)
            sel = p_act.tile([H_OTH, BT], dt.bfloat16, tag="sel")
            evac(EVAC["sel"], sel, selp)

            lp = psl.tile([LROWS, BT], dt.float32, tag="l", name="lp")
            vals = p_vals.tile([H_OTH, A, BT], dt.bfloat16, tag="vals")

            st = {"x2": x2, "lp": lp, "vals": vals, "b0": b0,
                  "sel": sel, "ots": ots}
            for a in range(A_SPLIT):
                agent_step(st, a)
            return st

        def agent_step(st, a):
            ots, sel, vals, lp = st["ots"], st["sel"], st["vals"], st["lp"]
            encp = ps1.tile([H_OTH, BT], dt.float32, tag="m", name="encp")
            nc.tensor.matmul(encp, s_we(a), ots[:, a, :],
                             start=True, stop=True)
            enc = p_enc.tile([H_OTH, BT], dt.bfloat16, tag="enc")
            evac(EVAC["enc"][a], enc, encp, bias=s_be[:, a:a + 1], relu=True)

            keysp = pskv.tile([H_OTH, BT], dt.float32, tag="kv",
                              name="keysp")
            nc.tensor.matmul(keysp, s_wk, enc, start=True, stop=True)
            valsp = pskv.tile([H_OTH, BT], dt.float32, tag="kv",
                              name="valsp")
            nc.tensor.matmul(valsp, s_wv, enc, start=True, stop=True)
            evac(EVAC["vals"][a], vals[:, a, :], valsp, bias=s_bv, relu=True)

            prod = p_prod.tile([H_OTH, BT], dt.bfloat16, tag="prod")
            nc.vector.tensor_mul(out=prod, in0=sel, in1=keysp)
            # segred: each agent's indicator lhsT sums prod's head blocks
            # into rows 8h+a of the shared [32, BT] logits tile (zeros
            # elsewhere; accumulation fills all 32 rows).
            nc.tensor.matmul(lp, s_sind(a), prod,
                             start=(a == 0), stop=False,
                             skip_group_check=True)

        def stage_a_back(st):
            for a in range(A_SPLIT, A):
                agent_step(st, a)

        def stage_b1a(st):
            """Softmax head: exp + segsum."""
            lp = st["lp"]
            ee = p_sm.tile([LROWS, BT], dt.bfloat16, tag="ee")
            nc.scalar.activation(ee, lp, AF.Exp)
            zp = psx3.tile([HEADS, BT], dt.float32, tag="x3", name="zp")
            nc.tensor.matmul(zp, s_t4, ee, start=True, stop=True)
            st["zp"] = zp

        def stage_b1(st):
            """Softmax tail: ln, broadcast-subtract, exp."""
            lp, zp = st["lp"], st["zp"]
            ls = p_sm.tile([HEADS, BT], dt.bfloat16, tag="ls")
            nc.scalar.activation(ls, zp, AF.Ln)
            nc.tensor.matmul(lp, s_negind, ls, start=False, stop=True,
                             skip_group_check=True)
            w = p_sm.tile([LROWS, BT], dt.bfloat16, tag="w")
            nc.scalar.activation(w, lp, AF.Exp)
            st["w"] = w

        def sadd(engine, out, in0, in1):
            if engine == "pool":
                nc.gpsimd.tensor_add(out, in0, in1)
            else:
                nc.vector.tensor_add(out, in0, in1)

        def stage_b2(st):
            """Per-agent broadcast matmul + product, w3o accumulation."""
            vals, w = st["vals"], st["w"]
            x3p = psx3.tile([H2, BT], dt.float32, tag="x3", name="x3p")
            nc.tensor.matmul(x3p, s_w3s, st["x2"], start=True, stop=False,
                             skip_group_check=True)
            for a in range(A):
                wbcp = pswb.tile([H_OTH, BT], dt.float32, tag="wb",
                                 name="wbcp")
                nc.tensor.matmul(wbcp, s_wl(a), w, start=True, stop=True)
                pa = p_pa.tile([H_OTH, BT], dt.bfloat16, tag="pa")
                nc.vector.tensor_mul(out=pa, in0=vals[:, a, :], in1=wbcp)
                nc.tensor.matmul(x3p, s_w3o, pa, start=False,
                                 stop=(a == A - 1), skip_group_check=True)
            st["x3p"] = x3p

        def stage_b3(st):
            x3p = st["x3p"]
            x3 = p_act.tile([H2, BT], dt.bfloat16, tag="x3s")
            nc.vector.tensor_scalar(out=x3, in0=x3p, scalar1=0.0,
                                    scalar2=None, op0=ALU.max)
            outp = psx3.tile([1, BT], dt.float32, tag="x3", name="outp")
            nc.tensor.matmul(outp, s_wout, x3, start=True, stop=True)
            nc.scalar.activation(outs_all[:, st["b0"]:st["b0"] + BT], outp,
                                 AF.Identity, bias=s_bout)

        # 2-stage software pipeline: tile t's B phases are emitted between
        # the two halves of tile t+1's stage A.
        prev = None
        prev2 = None
        prev3 = None
        for t in range(NTOT):
            if prev is not None:
                stage_b1a(prev)
            if prev3 is not None:
                stage_b3(prev3)
            stf = stage_a(t)
            if prev is not None:
                stage_b1(prev)
            stage_a_back(stf)
            if prev2 is not None:
                stage_b2(prev2)
            prev3 = prev2
            prev2 = prev
            prev = stf
        stage_b1a(prev)
        stage_b1(prev)
        stage_b2(prev2)
        stage_b3(prev3)
        stage_b3(prev2)
        stage_b2(prev)
        stage_b3(prev)

        nc.sync.dma_start(out=out_d.ap(), in_=outs_all)

    _split_sync_waits(nc)
    return nc


def _prep_inputs(state_one, act_one, state_others, act_others,
                 W1, b1, W2, b2, w3_self, We, be,
                 Wk, Wq, Wv, bv, w3_others, Wout, bout):
    """Host-side sharding + layout transforms. Returns per-core in_maps."""
    scale = 1.0 / np.sqrt(np.float32(AD))

    xt_full = np.concatenate([state_one, act_one], axis=1).T  # [78, B]
    inps = np.concatenate([state_others, act_others], axis=2)  # [A, B, 82]
    ot_full = np.zeros((OTH_IN, NA, B), dtype=BF16)
    ot_full[:, :A, :] = np.transpose(inps, (2, 0, 1))
    ot_full[:X_IN, A, :] = xt_full

    def headcat(wm):  # [H, J, AD] -> [J, H*AD]
        return np.ascontiguousarray(
            np.transpose(np.asarray(wm, np.float32), (1, 0, 2))
            .reshape(wm.shape[1], HEADS * AD))

    sind, t4, negind, wl = _indicator_constants()

    vals_bf16 = {
        "w1": np.asarray(W1, np.float32).astype(BF16),
        "w2": np.asarray(W2, np.float32).astype(BF16),
        "w3s": np.asarray(w3_self, np.float32).astype(BF16),
        "wq": (headcat(Wq) * scale).astype(BF16),
        "we": np.ascontiguousarray(
            np.transpose(np.asarray(We, np.float32), (1, 0, 2))
            .reshape(OTH_IN, A * H_OTH)).astype(BF16),
        "wk": headcat(Wk).astype(BF16),
        "wv": headcat(Wv).astype(BF16),
        "w3o": np.asarray(w3_others, np.float32).astype(BF16),
        "wout": np.asarray(Wout, np.float32).astype(BF16),
        "sind": sind.reshape(H_OTH, A * LROWS),
        "t4": t4,
        "negind": negind,
        "wl": wl.reshape(LROWS, A * H_OTH),
    }
    vals_f32 = {
        "b1": np.asarray(b1, np.float32).reshape(H_SELF, 1),
        "b2": np.asarray(b2, np.float32).reshape(H_SELF, 1),
        "be": np.ascontiguousarray(np.asarray(be, np.float32).T),
        "bv": np.asarray(bv, np.float32).reshape(HEADS * AD, 1),
        "bout": np.asarray(bout, np.float32).reshape(1, 1),
    }

    def pack(spec, values, dtype):
        off, width = _pack_layout(spec)
        arr = np.zeros((128, width), dtype=dtype)
        for name, rows, cols in spec:
            v = values[name]
            assert v.shape == (rows, cols), (name, v.shape, rows, cols)
            arr[:rows, off[name]:off[name] + cols] = v
        return arr

    cb = pack(_CONSTS_BF16, vals_bf16, BF16)
    cf = pack(_CONSTS_F32, vals_f32, np.float32)

    in_maps = []
    for c in range(NCORES):
        sl = slice(c * BC, (c + 1) * BC)
        m = {"cb": cb, "cf": cf,
             "ot": np.ascontiguousarray(ot_full[:, :, sl])}
        in_maps.append(m)
    return in_maps


def get_nc(reps=1):
    key = ("nc", reps)
    if key not in _CACHE:
        _CACHE[key] = _build_nc(reps)
    return _CACHE[key]


def kernel(**inputs) -> np.ndarray:
    from concourse.bass_utils import run_bass_kernel_spmd

    nc = get_nc()
    in_maps = _prep_inputs(**inputs)
    res = run_bass_kernel_spmd(nc, in_maps, core_ids=list(range(NCORES)))
    out = np.concatenate(
        [np.asarray(res.results[c]["out"], np.float32).reshape(BC, 1)
         for c in range(NCORES)], axis=0)
    return out

